# revision 1
# baseline (speedup 1.0000x reference)
"""Trainium2 Bass kernel for nn_AttentionBlock (B=2, C=512, L=64x64, 8 heads).

Sharding: 8 cores = 2 (batch) x 4 (head-groups of 2 heads each).
Each core: group-norm (replicated per batch), qkv for its 2 heads,
attention for its 2 heads, then AllGather of the per-head attention
output `a` within each batch's 4-core replica group, and a proj_out
row-slice (128 rows) + bias + residual.  Output per core: [128, 4096],
host concatenates to [2, 512, 64, 64].
"""

import sys

if "/opt/trn_rl_repo" not in sys.path:
    sys.path.insert(0, "/opt/trn_rl_repo")

import numpy as np

B, C = 2, 512
HW_L = 4096          # 64*64
NH, CHD, NG = 8, 64, 32
EPS = 1e-5
N_CORES = 8
SCALE = 1.0 / np.sqrt(np.sqrt(CHD))
TBLK = 512           # t-block (query block) size
GSZ = 3              # exp group size in psum banks (tiles of [128,512])
VARIANT = "full"     # timing-ablation switch; "full" is the real kernel
REPS = 1             # emit the whole body N times (differential timing)


def build_nc(L=HW_L):
    import concourse.bass as bass
    import concourse.tile as tile
    from concourse import bacc, mybir
    from contextlib import ExitStack

    f32 = mybir.dt.float32
    AF = mybir.ActivationFunctionType
    OP = mybir.AluOpType

    KT = C // 128           # 4 input-channel tiles
    NTB = L // TBLK         # t-blocks
    NS = L // 128           # s-tiles (key tiles)
    NCH = L // 512          # 512-wide chunks of L

    nc = bacc.Bacc("TRN2", target_bir_lowering=False, debug=False,
                   num_devices=N_CORES)

    x_ext = nc.dram_tensor("x", [C, L], f32, kind="ExternalInput")
    xres_ext = nc.dram_tensor("xres", [128, L], f32, kind="ExternalInput")
    wq_ext = nc.dram_tensor("wqT", [C, 128], f32, kind="ExternalInput")
    wk_ext = nc.dram_tensor("wkT", [C, 128], f32, kind="ExternalInput")
    wv_ext = nc.dram_tensor("wvT", [C, 128], f32, kind="ExternalInput")
    bq_ext = nc.dram_tensor("bq", [128, 1], f32, kind="ExternalInput")
    bk_ext = nc.dram_tensor("bk", [128, 1], f32, kind="ExternalInput")
    bv_ext = nc.dram_tensor("bv", [128, 1], f32, kind="ExternalInput")
    pw_ext = nc.dram_tensor("pwT", [C, 128], f32, kind="ExternalInput")
    pb_ext = nc.dram_tensor("pb", [128, 1], f32, kind="ExternalInput")
    nw_ext = nc.dram_tensor("nw", [C, 1], f32, kind="ExternalInput")
    nb_ext = nc.dram_tensor("nb", [C, 1], f32, kind="ExternalInput")
    gi_ext = nc.dram_tensor("gind", [NG, C], f32, kind="ExternalInput")
    giT_ext = nc.dram_tensor("gindT", [C, NG], f32, kind="ExternalInput")
    id_ext = nc.dram_tensor("ident", [128, 128], f32, kind="ExternalInput")
    out_ext = nc.dram_tensor("out", [128, L], f32, kind="ExternalOutput")

    with tile.TileContext(nc, num_cores=N_CORES) as tc, ExitStack() as ctx:
        pers = ctx.enter_context(tc.tile_pool(name="pers", bufs=1))
        accp = ctx.enter_context(
            tc.tile_pool(name="accp", bufs=2, space="PSUM"))
        dram = ctx.enter_context(tc.tile_pool(name="dram", bufs=1,
                                              space="DRAM"))

        # ---- constant / weight loads -------------------------------------
        wq_sb = [pers.tile([128, 128], f32, tag=f"wq{m}", name=f"wq{m}") for m in range(KT)]
        wk_sb = [pers.tile([128, 128], f32, tag=f"wk{m}", name=f"wk{m}") for m in range(KT)]
        wv_sb = [pers.tile([128, 128], f32, tag=f"wv{m}", name=f"wv{m}") for m in range(KT)]
        pw_sb = [pers.tile([128, 128], f32, tag=f"pw{m}", name=f"pw{m}") for m in range(KT)]
        for m in range(KT):
            nc.sync.dma_start(wq_sb[m][:], wq_ext[128 * m:128 * (m + 1), :])
            nc.sync.dma_start(wk_sb[m][:], wk_ext[128 * m:128 * (m + 1), :])
            nc.sync.dma_start(wv_sb[m][:], wv_ext[128 * m:128 * (m + 1), :])
            nc.sync.dma_start(pw_sb[m][:], pw_ext[128 * m:128 * (m + 1), :])
        bq_sb = pers.tile([128, 1], f32, tag="bq")
        bk_sb = pers.tile([128, 1], f32, tag="bk")
        bv_sb = pers.tile([128, 1], f32, tag="bv")
        pb_sb = pers.tile([128, 1], f32, tag="pb")
        nc.sync.dma_start(bq_sb[:], bq_ext[:])
        nc.sync.dma_start(bk_sb[:], bk_ext[:])
        nc.sync.dma_start(bv_sb[:], bv_ext[:])
        nc.sync.dma_start(pb_sb[:], pb_ext[:])
        w_part = [pers.tile([128, 1], f32, tag=f"nw{m}", name=f"nw{m}") for m in range(KT)]
        b_part = [pers.tile([128, 1], f32, tag=f"nb{m}", name=f"nb{m}") for m in range(KT)]
        for m in range(KT):
            nc.sync.dma_start(w_part[m][:], nw_ext[128 * m:128 * (m + 1), :])
            nc.sync.dma_start(b_part[m][:], nb_ext[128 * m:128 * (m + 1), :])
        gi_sb = pers.tile([NG, C], f32, tag="gi")
        nc.sync.dma_start(gi_sb[:], gi_ext[:])
        giT_sb = [pers.tile([128, NG], f32, tag=f"giT{m}", name=f"giT{m}") for m in range(KT)]
        for m in range(KT):
            nc.sync.dma_start(giT_sb[m][:], giT_ext[128 * m:128 * (m + 1), :])
        id_sb = pers.tile([128, 128], f32, tag="ident")
        nc.sync.dma_start(id_sb[:], id_ext[:])
        ones_sb = pers.tile([128, 64], f32, tag="ones")
        nc.gpsimd.memset(ones_sb[:], 1.0)

        xres_sb = pers.tile([128, L], f32, tag="xres")
        nc.sync.dma_start(xres_sb[:], xres_ext[:])
        # fold proj bias into the residual once
        xres_pb = pers.tile([128, L], f32, tag="xrespb")
        nc.vector.tensor_scalar_add(xres_pb[:], xres_sb[:], pb_sb[:])

        # persistent activation tensors
        q_sb = pers.tile([128, L], f32, tag="q")
        k_sb = pers.tile([128, L], f32, tag="k")
        v_sb = pers.tile([128, L], f32, tag="v")
        vT_sb = pers.tile([128, NS, 130], f32, tag="vT")
        nc.gpsimd.memset(vT_sb[:, :, 64:65], 1.0)
        nc.gpsimd.memset(vT_sb[:, :, 129:130], 1.0)

        def emit_body(rep):
          with tc.tile_pool(name=f"xpool{rep}", bufs=1) as xpool:
              xs = [xpool.tile([128, L], f32, tag=f"x{m}", name=f"x{m}") for m in range(KT)]
              stats = [xpool.tile([128, L // 512, 6], f32, tag=f"bs{m}", name=f"bs{m}")
                       for m in range(KT)]
              mv = [xpool.tile([128, 2], f32, tag=f"mv{m}", name=f"mv{m}") for m in range(KT)]
              rhs_m = [xpool.tile([128, 2], f32, tag=f"rh{m}", name=f"rh{m}")
                       for m in range(KT)]
              for m in range(KT):
                  for i in range(L // 512):
                      nc.sync.dma_start(
                          xs[m][:, 512 * i:512 * (i + 1)],
                          x_ext[128 * m:128 * (m + 1), 512 * i:512 * (i + 1)])
                      nc.vector.bn_stats(stats[m][:, i, :],
                                         xs[m][:, 512 * i:512 * (i + 1)])
                  nc.vector.bn_aggr(mv[m][:], stats[m][:])
                  # rhs_m = [mean, var + mean^2]
                  nc.vector.tensor_copy(rhs_m[m][:, 0:1], mv[m][:, 0:1])
                  nc.vector.tensor_tensor(rhs_m[m][:, 1:2], mv[m][:, 0:1],
                                          mv[m][:, 0:1], op=OP.mult)
                  nc.vector.tensor_tensor(rhs_m[m][:, 1:2], rhs_m[m][:, 1:2],
                                          mv[m][:, 1:2], op=OP.add)
              # group sums over channels: [32, 2] = sum_m giT[m].T @ rhs_m
              st32 = accp.tile([NG, 2], f32, tag="aps")
              for m in range(KT):
                  nc.tensor.matmul(st32[:], giT_sb[m][:], rhs_m[m][:],
                                   start=(m == 0), stop=(m == KT - 1))
              gstat = xpool.tile([NG, 2], f32, tag="gstat")
              nc.vector.tensor_scalar_mul(gstat[:], st32[:], 1.0 / 16.0)
              gvar = xpool.tile([NG, 1], f32, tag="gvar")
              nc.vector.tensor_tensor(gvar[:], gstat[:, 0:1], gstat[:, 0:1],
                                      op=OP.mult)
              nc.vector.tensor_tensor(gvar[:], gstat[:, 1:2], gvar[:],
                                      op=OP.subtract)
              # rstd = exp(-0.5 * ln(var + eps))  (Rsqrt ACT is banned)
              eps_sb = xpool.tile([NG, 1], f32, tag="eps")
              nc.gpsimd.memset(eps_sb[:], EPS)
              glog = xpool.tile([NG, 1], f32, tag="glog")
              nc.scalar.activation(glog[:], gvar[:], AF.Ln, bias=eps_sb[:])
              rstd = xpool.tile([NG, 1], f32, tag="rstd")
              nc.scalar.activation(rstd[:], glog[:], AF.Exp, scale=-0.5)

              s_part = [xpool.tile([128, 1], f32, tag=f"sp{m}", name=f"sp{m}")
                        for m in range(KT)]
              t_part = [xpool.tile([128, 1], f32, tag=f"tp{m}", name=f"tp{m}")
                        for m in range(KT)]
              ttmp = [xpool.tile([128, 1], f32, tag=f"tt{m}", name=f"tt{m}")
                      for m in range(KT)]
              for m in range(KT):
                  rc = accp.tile([128, 1], f32, tag="aps")
                  nc.tensor.matmul(rc[:], gi_sb[:, 128 * m:128 * (m + 1)],
                                   rstd[:])
                  mc = accp.tile([128, 1], f32, tag="aps")
                  nc.tensor.matmul(mc[:], gi_sb[:, 128 * m:128 * (m + 1)],
                                   gstat[:, 0:1])
                  nc.vector.tensor_tensor(s_part[m][:], w_part[m][:], rc[:],
                                          op=OP.mult)
                  nc.vector.tensor_tensor(ttmp[m][:], mc[:], s_part[m][:],
                                          op=OP.mult)
                  nc.vector.tensor_tensor(t_part[m][:], b_part[m][:],
                                          ttmp[m][:], op=OP.subtract)
                  # h = x * s + t  (in place over x)
                  nc.vector.tensor_scalar(xs[m][:], xs[m][:], s_part[m][:],
                                          t_part[m][:], op0=OP.mult, op1=OP.add)

              # ---- qkv projections -----------------------------------------
              for wt, bias, dest in ((wq_sb, bq_sb, q_sb),
                                     (wk_sb, bk_sb, k_sb),
                                     (wv_sb, bv_sb, v_sb)):
                  for nn in range(NCH):
                      ps = accp.tile([128, 512], f32, tag="aps")
                      for m in range(KT):
                          nc.tensor.matmul(
                              ps[:], wt[m][:],
                              xs[m][:, 512 * nn:512 * (nn + 1)],
                              start=(m == 0), stop=(m == KT - 1))
                      nc.vector.tensor_scalar_add(
                          dest[:, 512 * nn:512 * (nn + 1)], ps[:], bias[:])

              # ---- v^T (with ones columns at 64 and 129) --------------------
              for s in range(NS):
                  tp = accp.tile([128, 128], f32, tag="aps")
                  nc.tensor.transpose(tp[:], v_sb[:, 128 * s:128 * (s + 1)],
                                      id_sb[:])
                  nc.vector.tensor_copy(vT_sb[:, s, 0:64], tp[:, 0:64])
                  nc.vector.tensor_copy(vT_sb[:, s, 65:129], tp[:, 64:128])

          # ---- attention + gather + proj ---------------------------------
          NFLAT = 2 * NS
          groups = [list(range(i, min(i + GSZ, NFLAT)))
                      for i in range(0, NFLAT, GSZ)]
          a_tiles = {}

          with (
            tc.tile_pool(name=f"stp{rep}", bufs=2, space="PSUM") as stp,
            tc.tile_pool(name=f"expp{rep}", bufs=3) as expp,
            tc.tile_pool(name=f"attn{rep}", bufs=1) as attnp,
          ):
              def emit_half(h):
                  L2 = L // 2
                  agin = dram.tile([128, L2], f32, tag=f"agin{rep}_{h}")
                  nblk = L2 // TBLK
                  for j in range(nblk):
                      tb = h * nblk + j
                      ta, tbb = a_tiles[tb]
                      nc.sync.dma_start(
                          agin[0:64, TBLK * j:TBLK * (j + 1)], ta[:])
                      nc.sync.dma_start(
                          agin[64:128, TBLK * j:TBLK * (j + 1)], tbb[:])
                  agout = dram.tile([C, L2], f32, tag=f"agout{rep}_{h}")
                  if VARIANT == "nocoll":
                      nc.sync.dma_start(agout[0:128, :], agin[:])
                  else:
                      nc.gpsimd.collective_compute(
                          "AllGather", mybir.AluOpType.bypass,
                          replica_groups=[[0, 1, 2, 3], [4, 5, 6, 7]],
                          ins=[agin.opt()], outs=[agout.opt()])
                  return agout

              def emit_proj(h, agout):
                  L2 = L // 2
                  ag_sb = attnp.tile([128, KT, L2], f32, tag="ag", bufs=1)
                  nc.sync.dma_start(
                      ag_sb[:], agout[:].rearrange("(a p) f -> p a f", p=128))
                  nch = L2 // 512
                  for nn in range(nch):
                      ps = accp.tile([128, 512], f32, tag="aps")
                      for m in range(KT):
                          nc.tensor.matmul(
                              ps[:], pw_sb[m][:],
                              ag_sb[:, m, 512 * nn:512 * (nn + 1)],
                              start=(m == 0), stop=(m == KT - 1))
                      osb = attnp.tile([128, 512], f32, tag="osb", bufs=2)
                      col = h * L2 + 512 * nn
                      nc.vector.tensor_tensor(osb[:], ps[:],
                                              xres_pb[:, col:col + 512],
                                              op=OP.add)
                      nc.sync.dma_start(out_ext[:, col:col + 512], osb[:])

              half_ag = {}
              a_ps_cur = {}

              def emit_st(tb, grp):
                  t0 = TBLK * tb
                  stg = stp.tile([128, GSZ, TBLK], f32, tag="st")
                  for idx, f in enumerate(grp):
                      s, hd = f // 2, f % 2
                      nc.tensor.matmul(
                          stg[:, idx, :],
                          k_sb[64 * hd:64 * (hd + 1), 128 * s:128 * (s + 1)],
                          q_sb[64 * hd:64 * (hd + 1), t0:t0 + TBLK])
                  eg = expp.tile([128, GSZ, TBLK], f32, tag="eg")
                  nc.scalar.activation(eg[:, 0:len(grp), :],
                                       stg[:, 0:len(grp), :], AF.Exp)
                  return eg

              def emit_av(tb, grp, eg):
                  if tb not in a_ps_cur:
                      pA = accp.tile([65, TBLK], f32, tag="aps")
                      pB = accp.tile([65, TBLK], f32, tag="aps")
                      a_ps_cur[tb] = (pA, pB)
                  a_psA, a_psB = a_ps_cur[tb]
                  for idx, f in enumerate(grp):
                      s, hd = f // 2, f % 2
                      acc = a_psA if hd == 0 else a_psB
                      nc.tensor.matmul(
                          acc[:],
                          vT_sb[:, s, 65 * hd:65 * (hd + 1)],
                          eg[:, idx, :],
                          start=(s == 0), stop=(s == NS - 1),
                          skip_group_check=True)

              def emit_finalize(tb):
                  # normalize:  a[c,t] / denom[t];  denom sits in row 64.
                  # Copy everything out of the two aps PSUM slots FIRST so
                  # the next t-block's AV accumulators are not blocked
                  # behind the slow 1-lane reciprocal chain.
                  a_psA, a_psB = a_ps_cur.pop(tb)
                  dnA = attnp.tile([128, TBLK], f32, tag="rf", bufs=4)
                  nc.vector.tensor_copy(dnA[64:65, :], a_psA[64:65, :])
                  unA = attnp.tile([64, TBLK], f32, tag="un", bufs=2)
                  nc.vector.tensor_copy(unA[:], a_psA[0:64, :])
                  dnB = attnp.tile([128, TBLK], f32, tag="rf", bufs=4)
                  nc.vector.tensor_copy(dnB[64:65, :], a_psB[64:65, :])
                  unB = attnp.tile([64, TBLK], f32, tag="un", bufs=2)
                  nc.vector.tensor_copy(unB[:], a_psB[0:64, :])
                  # reciprocal on SBUF data (1-lane, slow, but off-PSUM);
                  # all ops stay on partition 64 (DVE lanes are fixed)
                  rfA = attnp.tile([128, TBLK], f32, tag="rf", bufs=4)
                  nc.vector.reciprocal(rfA[64:65, :], dnA[64:65, :])
                  rfB = attnp.tile([128, TBLK], f32, tag="rf", bufs=4)
                  nc.vector.reciprocal(rfB[64:65, :], dnB[64:65, :])
                  # broadcast tiles from the st pool (fast slot turnover)
                  rbA = stp.tile([64, TBLK], f32, tag="st")
                  nc.tensor.matmul(rbA[:], ones_sb[64:65, 0:64],
                                   rfA[64:65, :])
                  rbB = stp.tile([64, TBLK], f32, tag="st")
                  nc.tensor.matmul(rbB[:], ones_sb[64:65, 0:64],
                                   rfB[64:65, :])
                  ta = attnp.tile([64, TBLK], f32, tag="an", bufs=2 * NTB)
                  nc.vector.tensor_tensor(ta[:], unA[:], rbA[:], op=OP.mult)
                  tbb = attnp.tile([64, TBLK], f32, tag="an", bufs=2 * NTB)
                  nc.vector.tensor_tensor(tbb[:], unB[:], rbB[:], op=OP.mult)
                  a_tiles[tb] = (ta, tbb)
                  emit_post(tb)

              def emit_post(tb):
                  if VARIANT == "attnonly":
                      ta, tbb = a_tiles[tb]
                      col = TBLK * tb
                      nc.sync.dma_start(out_ext[0:64, col:col + TBLK], ta[:])
                      nc.sync.dma_start(out_ext[64:128, col:col + TBLK], tbb[:])
                      return
                  if tb == NTB // 2 - 1:
                      half_ag[0] = emit_half(0)
                  if tb == min(NTB // 2 + 1, NTB - 1):
                      emit_proj(0, half_ag[0])
                  if tb == NTB - 1:
                      half_ag[1] = emit_half(1)
                      emit_proj(1, half_ag[1])

              # software-pipelined by one group: the PE stream must not have
              # AV(g) (which waits on exp(g)) ahead of S^T(g+1), or the PE
              # stalls every group and the ACT engine (the bottleneck) idles.
              if VARIANT == "noattn":
                  for tb in range(NTB):
                      ta = attnp.tile([64, TBLK], f32, tag="an",
                                      bufs=2 * NTB, name=f"za{tb}")
                      nc.gpsimd.memset(ta[:], 0.5)
                      tbb = attnp.tile([64, TBLK], f32, tag="an",
                                       bufs=2 * NTB, name=f"zb{tb}")
                      nc.gpsimd.memset(tbb[:], 0.5)
                      a_tiles[tb] = (ta, tbb)
                      emit_post(tb)
              else:
                  flat = [(tb, grp) for tb in range(NTB) for grp in groups]
                  pend = None
                  for tb, grp in flat:
                      eg = emit_st(tb, grp)
                      if pend is not None:
                          ptb, pgrp, peg = pend
                          emit_av(ptb, pgrp, peg)
                          if pgrp is groups[-1]:
                              emit_finalize(ptb)
                      pend = (tb, grp, eg)
                  ptb, pgrp, peg = pend
                  emit_av(ptb, pgrp, peg)
                  emit_finalize(ptb)

        for rep in range(REPS):
            emit_body(rep)

    nc.compile()
    return nc


def prep_in_maps(inputs, L=HW_L):
    x = np.asarray(inputs["x"], dtype=np.float32).reshape(B, C, L)
    qkv_w = np.asarray(inputs["qkv_w"], dtype=np.float32)
    qkv_b = np.asarray(inputs["qkv_b"], dtype=np.float32)
    proj_w = np.asarray(inputs["proj_w"], dtype=np.float32)
    proj_b = np.asarray(inputs["proj_b"], dtype=np.float32)
    norm_w = np.asarray(inputs["norm_w"], dtype=np.float32)
    norm_b = np.asarray(inputs["norm_b"], dtype=np.float32)

    gind = np.zeros((NG, C), dtype=np.float32)
    gind[np.arange(C) // 16, np.arange(C)] = 1.0
    ident = np.eye(128, dtype=np.float32)

    def rows(h, kind):
        s = 192 * h + 64 * kind
        return slice(s, s + 64)

    in_maps = []
    for core in range(N_CORES):
        b, g = core // 4, core % 4
        hA, hB = 2 * g, 2 * g + 1
        wq = np.concatenate([qkv_w[rows(hA, 0)], qkv_w[rows(hB, 0)]], 0)
        wk = np.concatenate([qkv_w[rows(hA, 1)], qkv_w[rows(hB, 1)]], 0)
        wv = np.concatenate([qkv_w[rows(hA, 2)], qkv_w[rows(hB, 2)]], 0)
        bq = np.concatenate([qkv_b[rows(hA, 0)], qkv_b[rows(hB, 0)]])
        bk = np.concatenate([qkv_b[rows(hA, 1)], qkv_b[rows(hB, 1)]])
        bv = np.concatenate([qkv_b[rows(hA, 2)], qkv_b[rows(hB, 2)]])
        in_maps.append({
            "x": np.ascontiguousarray(x[b]),
            "xres": np.ascontiguousarray(x[b, 128 * g:128 * (g + 1), :]),
            "wqT": np.ascontiguousarray((SCALE * wq).T),
            "wkT": np.ascontiguousarray((SCALE * wk).T),
            "wvT": np.ascontiguousarray(wv.T),
            "bq": np.ascontiguousarray((SCALE * bq).reshape(128, 1)),
            "bk": np.ascontiguousarray((SCALE * bk).reshape(128, 1)),
            "bv": np.ascontiguousarray(bv.reshape(128, 1)),
            "pwT": np.ascontiguousarray(proj_w[128 * g:128 * (g + 1), :].T),
            "pb": np.ascontiguousarray(
                proj_b[128 * g:128 * (g + 1)].reshape(128, 1)),
            "nw": np.ascontiguousarray(norm_w.reshape(C, 1)),
            "nb": np.ascontiguousarray(norm_b.reshape(C, 1)),
            "gind": gind,
            "gindT": np.ascontiguousarray(gind.T),
            "ident": ident,
        })
    return in_maps


def gather_output(results, L=HW_L):
    out = np.empty((B, C, L), dtype=np.float32)
    for core in range(N_CORES):
        b, g = core // 4, core % 4
        out[b, 128 * g:128 * (g + 1), :] = results[core]["out"]
    s = int(np.sqrt(L))
    return out.reshape(B, C, s, s)


_NC_CACHE = {}


def get_nc(L=HW_L):
    if L not in _NC_CACHE:
        _NC_CACHE[L] = build_nc(L)
    return _NC_CACHE[L]


def kernel(**inputs):
    from concourse.bass_utils import run_bass_kernel_spmd

    nc = get_nc()
    in_maps = prep_in_maps(inputs)
    res = run_bass_kernel_spmd(nc, in_maps, core_ids=list(range(N_CORES)))
    return gather_output(res.results)


if __name__ == "__main__":
    rng = np.random.default_rng(0)
    inputs = {
        "x": rng.standard_normal((B, C, 64, 64), dtype=np.float32),
        "norm_w": rng.standard_normal(C, dtype=np.float32) * 0.1 + 1.0,
        "norm_b": rng.standard_normal(C, dtype=np.float32) * 0.1,
        "qkv_w": (rng.standard_normal((3 * C, C), dtype=np.float32)
                  / np.sqrt(C)),
        "qkv_b": rng.standard_normal(3 * C, dtype=np.float32) * 0.02,
        "proj_w": (rng.standard_normal((C, C), dtype=np.float32)
                   / np.sqrt(C)),
        "proj_b": rng.standard_normal(C, dtype=np.float32) * 0.02,
    }
    out = kernel(**inputs)
    print("kernel output", out.shape, out.dtype, float(np.abs(out).mean()))



# revision 9
# speedup vs baseline: 2.0404x; 2.0404x over previous
"""Trainium2 Bass kernel for nn_AttentionBlock (B=2, C=512, L=64x64, 8 heads).

Sharding: 8 cores = 2 (batch) x 4 (head-groups of 2 heads each).
Each core: group-norm stats (replicated per batch), qkv for its 2 heads,
attention for its 2 heads, then AllGather of the per-head attention
output `a` within each batch's 4-core replica group, and a proj_out
row-slice (128 rows) + bias + residual.  Output per core: [128, 4096],
host concatenates to [2, 512, 64, 64].

All large matmuls run in bf16 (1 PE cycle/row vs 4 for fp32); the
group-norm scale/shift is folded into the qkv weights+biases
(W @ (x*s+t) = (W.s) @ x + (b + W@t)) so the bf16 x feeds the matmul
straight from DMA.  Residual + proj bias stay fp32.
"""

import sys

if "/opt/trn_rl_repo" not in sys.path:
    sys.path.insert(0, "/opt/trn_rl_repo")

import numpy as np
import ml_dtypes

B, C = 2, 512
HW_L = 4096          # 64*64
NH, CHD, NG = 8, 64, 32
EPS = 1e-5
N_CORES = 8
SCALE = 1.0 / np.sqrt(np.sqrt(CHD))
TBLK = 512           # t-block (query block) size
GSZ = 3              # exp group size in psum banks (tiles of [128,512])
VARIANT = "full"     # timing-ablation switch; "full" is the real kernel
REPS = 1             # emit the whole body N times (differential timing)
BF16 = ml_dtypes.bfloat16


def build_nc(L=HW_L):
    import concourse.bass as bass
    import concourse.tile as tile
    from concourse import bacc, mybir
    from contextlib import ExitStack

    f32 = mybir.dt.float32
    bf16 = mybir.dt.bfloat16
    AF = mybir.ActivationFunctionType
    OP = mybir.AluOpType

    KT = C // 128           # 4 input-channel tiles
    NTB = L // TBLK         # t-blocks
    NS = L // 128           # s-tiles (key tiles)
    NCH = L // 512          # 512-wide chunks of L

    nc = bacc.Bacc("TRN2", target_bir_lowering=False, debug=False,
                   num_devices=N_CORES)

    x_ext = nc.dram_tensor("x", [C, L], bf16, kind="ExternalInput")
    xres_ext = nc.dram_tensor("xres", [128, L], f32, kind="ExternalInput")
    wq_ext = nc.dram_tensor("wqT", [C, 128], f32, kind="ExternalInput")
    wk_ext = nc.dram_tensor("wkT", [C, 128], f32, kind="ExternalInput")
    wv_ext = nc.dram_tensor("wvT", [C, 128], f32, kind="ExternalInput")
    bq_ext = nc.dram_tensor("bq", [128, 1], f32, kind="ExternalInput")
    bk_ext = nc.dram_tensor("bk", [128, 1], f32, kind="ExternalInput")
    bv_ext = nc.dram_tensor("bv", [128, 1], f32, kind="ExternalInput")
    pw_ext = nc.dram_tensor("pwT", [C, 128], bf16, kind="ExternalInput")
    pb_ext = nc.dram_tensor("pb", [128, 1], f32, kind="ExternalInput")
    nw_ext = nc.dram_tensor("nw", [C, 1], f32, kind="ExternalInput")
    nb_ext = nc.dram_tensor("nb", [C, 1], f32, kind="ExternalInput")
    gi_ext = nc.dram_tensor("gind", [NG, C], f32, kind="ExternalInput")
    giT_ext = nc.dram_tensor("gindT", [C, NG], f32, kind="ExternalInput")
    id_ext = nc.dram_tensor("identb", [128, 128], bf16, kind="ExternalInput")
    out_ext = nc.dram_tensor("out", [128, L], f32, kind="ExternalOutput")

    with tile.TileContext(nc, num_cores=N_CORES) as tc, ExitStack() as ctx:
        pers = ctx.enter_context(tc.tile_pool(name="pers", bufs=1))
        accp = ctx.enter_context(
            tc.tile_pool(name="accp", bufs=2, space="PSUM"))
        dram = ctx.enter_context(tc.tile_pool(name="dram", bufs=1,
                                              space="DRAM"))

        # ---- constant / weight loads -------------------------------------
        wq_sb = [pers.tile([128, 128], f32, tag=f"wq{m}", name=f"wq{m}") for m in range(KT)]
        wk_sb = [pers.tile([128, 128], f32, tag=f"wk{m}", name=f"wk{m}") for m in range(KT)]
        wv_sb = [pers.tile([128, 128], f32, tag=f"wv{m}", name=f"wv{m}") for m in range(KT)]
        pw_sb = [pers.tile([128, 128], bf16, tag=f"pw{m}", name=f"pw{m}") for m in range(KT)]
        for m in range(KT):
            nc.sync.dma_start(wq_sb[m][:], wq_ext[128 * m:128 * (m + 1), :])
            nc.sync.dma_start(wk_sb[m][:], wk_ext[128 * m:128 * (m + 1), :])
            nc.sync.dma_start(wv_sb[m][:], wv_ext[128 * m:128 * (m + 1), :])
            nc.sync.dma_start(pw_sb[m][:], pw_ext[128 * m:128 * (m + 1), :])
        bq_sb = pers.tile([128, 1], f32, tag="bq")
        bk_sb = pers.tile([128, 1], f32, tag="bk")
        bv_sb = pers.tile([128, 1], f32, tag="bv")
        pb_sb = pers.tile([128, 1], f32, tag="pb")
        nc.sync.dma_start(bq_sb[:], bq_ext[:])
        nc.sync.dma_start(bk_sb[:], bk_ext[:])
        nc.sync.dma_start(bv_sb[:], bv_ext[:])
        nc.sync.dma_start(pb_sb[:], pb_ext[:])
        w_part = [pers.tile([128, 1], f32, tag=f"nw{m}", name=f"nw{m}") for m in range(KT)]
        b_part = [pers.tile([128, 1], f32, tag=f"nb{m}", name=f"nb{m}") for m in range(KT)]
        for m in range(KT):
            nc.sync.dma_start(w_part[m][:], nw_ext[128 * m:128 * (m + 1), :])
            nc.sync.dma_start(b_part[m][:], nb_ext[128 * m:128 * (m + 1), :])
        gi_sb = pers.tile([NG, C], f32, tag="gi")
        nc.sync.dma_start(gi_sb[:], gi_ext[:])
        giT_sb = [pers.tile([128, NG], f32, tag=f"giT{m}", name=f"giT{m}") for m in range(KT)]
        for m in range(KT):
            nc.sync.dma_start(giT_sb[m][:], giT_ext[128 * m:128 * (m + 1), :])
        id_sb = pers.tile([128, 128], bf16, tag="ident")
        nc.sync.dma_start(id_sb[:], id_ext[:])
        ones_sb = pers.tile([128, 64], bf16, tag="ones")
        nc.gpsimd.memset(ones_sb[:], 1.0)

        xres_sb = pers.tile([128, L], f32, tag="xres")
        nc.sync.dma_start(xres_sb[:], xres_ext[:])
        # fold proj bias into the residual once
        xres_pb = pers.tile([128, L], f32, tag="xrespb")
        nc.vector.tensor_scalar_add(xres_pb[:], xres_sb[:], pb_sb[:])

        # persistent activation tensors
        q_sb = pers.tile([128, L], bf16, tag="q")
        k_sb = pers.tile([128, L], bf16, tag="k")
        v_sb = pers.tile([128, L], bf16, tag="v")
        vT_sb = pers.tile([128, NS, 130], bf16, tag="vT")
        nc.gpsimd.memset(vT_sb[:, :, 64:65], 1.0)
        nc.gpsimd.memset(vT_sb[:, :, 129:130], 1.0)

        def emit_body(rep):
          with tc.tile_pool(name=f"xpool{rep}", bufs=1) as xpool:
              xs = [xpool.tile([128, L], bf16, tag=f"x{m}", name=f"x{m}") for m in range(KT)]
              stats = [xpool.tile([128, L // 512, 6], f32, tag=f"bs{m}", name=f"bs{m}")
                       for m in range(KT)]
              mv = [xpool.tile([128, 2], f32, tag=f"mv{m}", name=f"mv{m}") for m in range(KT)]
              rhs_m = [xpool.tile([128, 2], f32, tag=f"rh{m}", name=f"rh{m}")
                       for m in range(KT)]
              for m in range(KT):
                  for i in range(L // 512):
                      nc.sync.dma_start(
                          xs[m][:, 512 * i:512 * (i + 1)],
                          x_ext[128 * m:128 * (m + 1), 512 * i:512 * (i + 1)])
                      nc.vector.bn_stats(stats[m][:, i, :],
                                         xs[m][:, 512 * i:512 * (i + 1)])
                  nc.vector.bn_aggr(mv[m][:], stats[m][:])
                  # rhs_m = [mean, var + mean^2]
                  nc.vector.tensor_copy(rhs_m[m][:, 0:1], mv[m][:, 0:1])
                  nc.vector.tensor_tensor(rhs_m[m][:, 1:2], mv[m][:, 0:1],
                                          mv[m][:, 0:1], op=OP.mult)
                  nc.vector.tensor_tensor(rhs_m[m][:, 1:2], rhs_m[m][:, 1:2],
                                          mv[m][:, 1:2], op=OP.add)
              # group sums over channels: [32, 2] = sum_m giT[m].T @ rhs_m
              st32 = accp.tile([NG, 2], f32, tag="aps")
              for m in range(KT):
                  nc.tensor.matmul(st32[:], giT_sb[m][:], rhs_m[m][:],
                                   start=(m == 0), stop=(m == KT - 1))
              gstat = xpool.tile([NG, 2], f32, tag="gstat")
              nc.vector.tensor_scalar_mul(gstat[:], st32[:], 1.0 / 16.0)
              gvar = xpool.tile([NG, 1], f32, tag="gvar")
              nc.vector.tensor_tensor(gvar[:], gstat[:, 0:1], gstat[:, 0:1],
                                      op=OP.mult)
              nc.vector.tensor_tensor(gvar[:], gstat[:, 1:2], gvar[:],
                                      op=OP.subtract)
              # rstd = exp(-0.5 * ln(var + eps))  (Rsqrt ACT is banned)
              eps_sb = xpool.tile([NG, 1], f32, tag="eps")
              nc.gpsimd.memset(eps_sb[:], EPS)
              glog = xpool.tile([NG, 1], f32, tag="glog")
              nc.scalar.activation(glog[:], gvar[:], AF.Ln, bias=eps_sb[:])
              rstd = xpool.tile([NG, 1], f32, tag="rstd")
              nc.scalar.activation(rstd[:], glog[:], AF.Exp, scale=-0.5)

              s_part = [xpool.tile([128, 1], f32, tag=f"sp{m}", name=f"sp{m}")
                        for m in range(KT)]
              t_part = [xpool.tile([128, 1], f32, tag=f"tp{m}", name=f"tp{m}")
                        for m in range(KT)]
              ttmp = [xpool.tile([128, 1], f32, tag=f"tt{m}", name=f"tt{m}")
                      for m in range(KT)]
              for m in range(KT):
                  rc = accp.tile([128, 1], f32, tag="aps")
                  nc.tensor.matmul(rc[:], gi_sb[:, 128 * m:128 * (m + 1)],
                                   rstd[:])
                  mc = accp.tile([128, 1], f32, tag="aps")
                  nc.tensor.matmul(mc[:], gi_sb[:, 128 * m:128 * (m + 1)],
                                   gstat[:, 0:1])
                  nc.vector.tensor_tensor(s_part[m][:], w_part[m][:], rc[:],
                                          op=OP.mult)
                  nc.vector.tensor_tensor(ttmp[m][:], mc[:], s_part[m][:],
                                          op=OP.mult)
                  nc.vector.tensor_tensor(t_part[m][:], b_part[m][:],
                                          ttmp[m][:], op=OP.subtract)

              # ---- fold group-norm into qkv weights + biases ---------------
              # q = (W.s) @ x + (b + W @ t), with W already SCALE-scaled on
              # the host for q/k.
              wqs = [xpool.tile([128, 128], bf16, tag=f"wqs{m}", name=f"wqs{m}")
                     for m in range(KT)]
              wks = [xpool.tile([128, 128], bf16, tag=f"wks{m}", name=f"wks{m}")
                     for m in range(KT)]
              wvs = [xpool.tile([128, 128], bf16, tag=f"wvs{m}", name=f"wvs{m}")
                     for m in range(KT)]
              for m in range(KT):
                  nc.vector.tensor_scalar_mul(wqs[m][:], wq_sb[m][:],
                                              s_part[m][:])
                  nc.vector.tensor_scalar_mul(wks[m][:], wk_sb[m][:],
                                              s_part[m][:])
                  nc.vector.tensor_scalar_mul(wvs[m][:], wv_sb[m][:],
                                              s_part[m][:])
              badj = {}
              for nm, wt, bias in (("q", wq_sb, bq_sb), ("k", wk_sb, bk_sb),
                                   ("v", wv_sb, bv_sb)):
                  bb = accp.tile([128, 1], f32, tag="aps")
                  for m in range(KT):
                      nc.tensor.matmul(bb[:], wt[m][:], t_part[m][:],
                                       start=(m == 0), stop=(m == KT - 1))
                  ba = xpool.tile([128, 1], f32, tag=f"badj{nm}")
                  nc.vector.tensor_tensor(ba[:], bias[:], bb[:], op=OP.add)
                  badj[nm] = ba

              # ---- qkv projections -----------------------------------------
              for wt, nm, dest in ((wqs, "q", q_sb),
                                   (wks, "k", k_sb),
                                   (wvs, "v", v_sb)):
                  for nn in range(NCH):
                      ps = accp.tile([128, 512], f32, tag="aps")
                      for m in range(KT):
                          nc.tensor.matmul(
                              ps[:], wt[m][:],
                              xs[m][:, 512 * nn:512 * (nn + 1)],
                              start=(m == 0), stop=(m == KT - 1))
                      nc.vector.tensor_scalar_add(
                          dest[:, 512 * nn:512 * (nn + 1)], ps[:],
                          badj[nm][:])

              # ---- v^T (with ones columns at 64 and 129) --------------------
              for s in range(NS):
                  tp = accp.tile([128, 128], bf16, tag="aps")
                  nc.tensor.transpose(tp[:], v_sb[:, 128 * s:128 * (s + 1)],
                                      id_sb[:])
                  nc.vector.tensor_copy(vT_sb[:, s, 0:64], tp[:, 0:64])
                  nc.vector.tensor_copy(vT_sb[:, s, 65:129], tp[:, 64:128])

          # ---- attention + gather + proj ---------------------------------
          NFLAT = 2 * NS
          groups = [list(range(i, min(i + GSZ, NFLAT)))
                      for i in range(0, NFLAT, GSZ)]
          a_tiles = {}

          with (
            tc.tile_pool(name=f"stp{rep}", bufs=2, space="PSUM") as stp,
            tc.tile_pool(name=f"expp{rep}", bufs=3) as expp,
            tc.tile_pool(name=f"attn{rep}", bufs=1) as attnp,
          ):
              def emit_half(h):
                  L2 = L // 2
                  agin = dram.tile([128, L2], bf16, tag=f"agin{rep}_{h}")
                  nblk = L2 // TBLK
                  for j in range(nblk):
                      tb = h * nblk + j
                      ta, tbb = a_tiles[tb]
                      nc.sync.dma_start(
                          agin[0:64, TBLK * j:TBLK * (j + 1)], ta[:])
                      nc.sync.dma_start(
                          agin[64:128, TBLK * j:TBLK * (j + 1)], tbb[:])
                  agout = dram.tile([C, L2], bf16, tag=f"agout{rep}_{h}")
                  if VARIANT == "nocoll":
                      nc.sync.dma_start(agout[0:128, :], agin[:])
                  else:
                      nc.gpsimd.collective_compute(
                          "AllGather", mybir.AluOpType.bypass,
                          replica_groups=[[0, 1, 2, 3], [4, 5, 6, 7]],
                          ins=[agin.opt()], outs=[agout.opt()])
                  return agout

              def emit_proj(h, agout):
                  L2 = L // 2
                  ag_sb = attnp.tile([128, KT, L2], bf16, tag="ag", bufs=1)
                  nc.sync.dma_start(
                      ag_sb[:], agout[:].rearrange("(a p) f -> p a f", p=128))
                  nch = L2 // 512
                  for nn in range(nch):
                      ps = accp.tile([128, 512], f32, tag="aps")
                      for m in range(KT):
                          nc.tensor.matmul(
                              ps[:], pw_sb[m][:],
                              ag_sb[:, m, 512 * nn:512 * (nn + 1)],
                              start=(m == 0), stop=(m == KT - 1))
                      osb = attnp.tile([128, 512], f32, tag="osb", bufs=2)
                      col = h * L2 + 512 * nn
                      nc.vector.tensor_tensor(osb[:], ps[:],
                                              xres_pb[:, col:col + 512],
                                              op=OP.add)
                      nc.sync.dma_start(out_ext[:, col:col + 512], osb[:])

              half_ag = {}
              a_ps_cur = {}

              def emit_st(tb, grp):
                  t0 = TBLK * tb
                  stg = stp.tile([128, GSZ, TBLK], f32, tag="st")
                  for idx, f in enumerate(grp):
                      s, hd = f // 2, f % 2
                      nc.tensor.matmul(
                          stg[:, idx, :],
                          k_sb[64 * hd:64 * (hd + 1), 128 * s:128 * (s + 1)],
                          q_sb[64 * hd:64 * (hd + 1), t0:t0 + TBLK])
                  eg = expp.tile([128, GSZ, TBLK], bf16, tag="eg")
                  nc.scalar.activation(eg[:, 0:len(grp), :],
                                       stg[:, 0:len(grp), :], AF.Exp)
                  return eg

              def emit_av(tb, grp, eg):
                  if tb not in a_ps_cur:
                      pA = accp.tile([65, TBLK], f32, tag="aps")
                      pB = accp.tile([65, TBLK], f32, tag="aps")
                      a_ps_cur[tb] = (pA, pB)
                  a_psA, a_psB = a_ps_cur[tb]
                  for idx, f in enumerate(grp):
                      s, hd = f // 2, f % 2
                      acc = a_psA if hd == 0 else a_psB
                      nc.tensor.matmul(
                          acc[:],
                          vT_sb[:, s, 65 * hd:65 * (hd + 1)],
                          eg[:, idx, :],
                          start=(s == 0), stop=(s == NS - 1),
                          skip_group_check=True)

              def emit_finalize(tb):
                  # normalize:  a[c,t] / denom[t];  denom sits in row 64.
                  # Copy everything out of the two aps PSUM slots FIRST so
                  # the next t-block's AV accumulators are not blocked
                  # behind the slow 1-lane reciprocal chain.
                  a_psA, a_psB = a_ps_cur.pop(tb)
                  dnA = attnp.tile([128, TBLK], f32, tag="rf", bufs=4)
                  nc.vector.tensor_copy(dnA[64:65, :], a_psA[64:65, :])
                  unA = attnp.tile([64, TBLK], f32, tag="un", bufs=2)
                  nc.vector.tensor_copy(unA[:], a_psA[0:64, :])
                  dnB = attnp.tile([128, TBLK], f32, tag="rf", bufs=4)
                  nc.vector.tensor_copy(dnB[64:65, :], a_psB[64:65, :])
                  unB = attnp.tile([64, TBLK], f32, tag="un", bufs=2)
                  nc.vector.tensor_copy(unB[:], a_psB[0:64, :])
                  # reciprocal on SBUF data (1-lane, slow, but off-PSUM);
                  # all ops stay on partition 64 (DVE lanes are fixed)
                  rfA = attnp.tile([128, TBLK], bf16, tag="rfb", bufs=4)
                  rfB = attnp.tile([128, TBLK], bf16, tag="rfb", bufs=4)
                  with nc.allow_low_precision(
                          reason="softmax denom reciprocal in bf16; "
                                 "0.4% rel err is well inside tolerance"):
                      nc.vector.reciprocal(rfA[64:65, :], dnA[64:65, :])
                      nc.vector.reciprocal(rfB[64:65, :], dnB[64:65, :])
                  # broadcast tiles from the st pool (fast slot turnover)
                  rbA = stp.tile([64, TBLK], f32, tag="st")
                  nc.tensor.matmul(rbA[:], ones_sb[64:65, 0:64],
                                   rfA[64:65, :])
                  rbB = stp.tile([64, TBLK], f32, tag="st")
                  nc.tensor.matmul(rbB[:], ones_sb[64:65, 0:64],
                                   rfB[64:65, :])
                  ta = attnp.tile([64, TBLK], bf16, tag="an", bufs=2 * NTB)
                  nc.vector.tensor_tensor(ta[:], unA[:], rbA[:], op=OP.mult)
                  tbb = attnp.tile([64, TBLK], bf16, tag="an", bufs=2 * NTB)
                  nc.vector.tensor_tensor(tbb[:], unB[:], rbB[:], op=OP.mult)
                  a_tiles[tb] = (ta, tbb)
                  emit_post(tb)

              def emit_post(tb):
                  if VARIANT == "attnonly":
                      ta, tbb = a_tiles[tb]
                      col = TBLK * tb
                      nc.sync.dma_start(out_ext[0:64, col:col + TBLK], ta[:])
                      nc.sync.dma_start(out_ext[64:128, col:col + TBLK], tbb[:])
                      return
                  if tb == NTB // 2 - 1:
                      half_ag[0] = emit_half(0)
                  if tb == min(NTB // 2 + 1, NTB - 1):
                      emit_proj(0, half_ag[0])
                  if tb == NTB - 1:
                      half_ag[1] = emit_half(1)
                      emit_proj(1, half_ag[1])

              # software-pipelined by one group: the PE stream must not have
              # AV(g) (which waits on exp(g)) ahead of S^T(g+1), or the PE
              # stalls every group and the ACT engine (the bottleneck) idles.
              if VARIANT == "noattn":
                  for tb in range(NTB):
                      ta = attnp.tile([64, TBLK], bf16, tag="an",
                                      bufs=2 * NTB, name=f"za{tb}")
                      nc.gpsimd.memset(ta[:], 0.5)
                      tbb = attnp.tile([64, TBLK], bf16, tag="an",
                                       bufs=2 * NTB, name=f"zb{tb}")
                      nc.gpsimd.memset(tbb[:], 0.5)
                      a_tiles[tb] = (ta, tbb)
                      emit_post(tb)
              else:
                  flat = [(tb, grp) for tb in range(NTB) for grp in groups]
                  pend = None
                  for tb, grp in flat:
                      eg = emit_st(tb, grp)
                      if pend is not None:
                          ptb, pgrp, peg = pend
                          emit_av(ptb, pgrp, peg)
                          if pgrp is groups[-1]:
                              emit_finalize(ptb)
                      pend = (tb, grp, eg)
                  ptb, pgrp, peg = pend
                  emit_av(ptb, pgrp, peg)
                  emit_finalize(ptb)

        for rep in range(REPS):
            emit_body(rep)

    nc.compile()
    return nc


def prep_in_maps(inputs, L=HW_L):
    x = np.asarray(inputs["x"], dtype=np.float32).reshape(B, C, L)
    qkv_w = np.asarray(inputs["qkv_w"], dtype=np.float32)
    qkv_b = np.asarray(inputs["qkv_b"], dtype=np.float32)
    proj_w = np.asarray(inputs["proj_w"], dtype=np.float32)
    proj_b = np.asarray(inputs["proj_b"], dtype=np.float32)
    norm_w = np.asarray(inputs["norm_w"], dtype=np.float32)
    norm_b = np.asarray(inputs["norm_b"], dtype=np.float32)

    gind = np.zeros((NG, C), dtype=np.float32)
    gind[np.arange(C) // 16, np.arange(C)] = 1.0
    identb = np.eye(128, dtype=np.float32).astype(BF16)

    def rows(h, kind):
        s = 192 * h + 64 * kind
        return slice(s, s + 64)

    in_maps = []
    for core in range(N_CORES):
        b, g = core // 4, core % 4
        hA, hB = 2 * g, 2 * g + 1
        wq = np.concatenate([qkv_w[rows(hA, 0)], qkv_w[rows(hB, 0)]], 0)
        wk = np.concatenate([qkv_w[rows(hA, 1)], qkv_w[rows(hB, 1)]], 0)
        wv = np.concatenate([qkv_w[rows(hA, 2)], qkv_w[rows(hB, 2)]], 0)
        bq = np.concatenate([qkv_b[rows(hA, 0)], qkv_b[rows(hB, 0)]])
        bk = np.concatenate([qkv_b[rows(hA, 1)], qkv_b[rows(hB, 1)]])
        bv = np.concatenate([qkv_b[rows(hA, 2)], qkv_b[rows(hB, 2)]])
        in_maps.append({
            "x": np.ascontiguousarray(x[b]).astype(BF16),
            "xres": np.ascontiguousarray(x[b, 128 * g:128 * (g + 1), :]),
            "wqT": np.ascontiguousarray((SCALE * wq).T),
            "wkT": np.ascontiguousarray((SCALE * wk).T),
            "wvT": np.ascontiguousarray(wv.T),
            "bq": np.ascontiguousarray((SCALE * bq).reshape(128, 1)),
            "bk": np.ascontiguousarray((SCALE * bk).reshape(128, 1)),
            "bv": np.ascontiguousarray(bv.reshape(128, 1)),
            "pwT": np.ascontiguousarray(
                proj_w[128 * g:128 * (g + 1), :].T).astype(BF16),
            "pb": np.ascontiguousarray(
                proj_b[128 * g:128 * (g + 1)].reshape(128, 1)),
            "nw": np.ascontiguousarray(norm_w.reshape(C, 1)),
            "nb": np.ascontiguousarray(norm_b.reshape(C, 1)),
            "gind": gind,
            "gindT": np.ascontiguousarray(gind.T),
            "identb": identb,
        })
    return in_maps


def gather_output(results, L=HW_L):
    out = np.empty((B, C, L), dtype=np.float32)
    for core in range(N_CORES):
        b, g = core // 4, core % 4
        out[b, 128 * g:128 * (g + 1), :] = results[core]["out"]
    s = int(np.sqrt(L))
    return out.reshape(B, C, s, s)


_NC_CACHE = {}


def get_nc(L=HW_L):
    if L not in _NC_CACHE:
        _NC_CACHE[L] = build_nc(L)
    return _NC_CACHE[L]


def kernel(**inputs):
    from concourse.bass_utils import run_bass_kernel_spmd

    nc = get_nc()
    in_maps = prep_in_maps(inputs)
    res = run_bass_kernel_spmd(nc, in_maps, core_ids=list(range(N_CORES)))
    return gather_output(res.results)


if __name__ == "__main__":
    rng = np.random.default_rng(0)
    inputs = {
        "x": rng.standard_normal((B, C, 64, 64), dtype=np.float32),
        "norm_w": rng.standard_normal(C, dtype=np.float32) * 0.1 + 1.0,
        "norm_b": rng.standard_normal(C, dtype=np.float32) * 0.1,
        "qkv_w": (rng.standard_normal((3 * C, C), dtype=np.float32)
                  / np.sqrt(C)),
        "qkv_b": rng.standard_normal(3 * C, dtype=np.float32) * 0.02,
        "proj_w": (rng.standard_normal((C, C), dtype=np.float32)
                   / np.sqrt(C)),
        "proj_b": rng.standard_normal(C, dtype=np.float32) * 0.02,
    }
    out = kernel(**inputs)
    print("kernel output", out.shape, out.dtype, float(np.abs(out).mean()))


# revision 17
# speedup vs baseline: 2.8438x; 1.3938x over previous
"""Trainium2 Bass kernel for nn_AttentionBlock (B=2, C=512, L=64x64, 8 heads).

Sharding: 8 cores = 2 (batch) x 4 (head-groups of 2 heads each).
Each core: group-norm stats (replicated per batch), qkv for its 2 heads,
attention for its 2 heads, then AllGather of the per-head attention
output `a` within each batch's 4-core replica group, and a proj_out
row-slice (128 rows) + bias + residual.  Output per core: [128, 4096],
host concatenates to [2, 512, 64, 64].

All large matmuls run in bf16 (1 PE cycle/row vs 4 for fp32); the
group-norm scale/shift is folded into the qkv weights+biases
(W @ (x*s+t) = (W.s) @ x + (b + W@t)) so the bf16 x feeds the matmul
straight from DMA.  Residual + proj bias stay fp32.
"""

import sys

if "/opt/trn_rl_repo" not in sys.path:
    sys.path.insert(0, "/opt/trn_rl_repo")

import numpy as np
import ml_dtypes

B, C = 2, 512
HW_L = 4096          # 64*64
NH, CHD, NG = 8, 64, 32
EPS = 1e-5
N_CORES = 8
SCALE = 1.0 / np.sqrt(np.sqrt(CHD))
TBLK = 512           # t-block (query block) size
GSZ = 3              # exp group size in psum banks (tiles of [128,512])
VARIANT = "full"     # timing-ablation switch; "full" is the real kernel
REPS = 1             # emit the whole body N times (differential timing)
BF16 = ml_dtypes.bfloat16


def build_nc(L=HW_L):
    import concourse.bass as bass
    import concourse.tile as tile
    from concourse import bacc, mybir
    from contextlib import ExitStack

    f32 = mybir.dt.float32
    bf16 = mybir.dt.bfloat16
    AF = mybir.ActivationFunctionType
    OP = mybir.AluOpType

    KT = C // 128           # 4 input-channel tiles
    NTB = L // TBLK         # t-blocks
    NS = L // 128           # s-tiles (key tiles)
    NCH = L // 512          # 512-wide chunks of L

    nc = bacc.Bacc("TRN2", target_bir_lowering=False, debug=False,
                   num_devices=N_CORES)

    x_ext = nc.dram_tensor("x", [C, L], bf16, kind="ExternalInput")
    xres_ext = nc.dram_tensor("xres", [128, L], f32, kind="ExternalInput")
    wq_ext = nc.dram_tensor("wqT", [C, 128], f32, kind="ExternalInput")
    wk_ext = nc.dram_tensor("wkT", [C, 128], f32, kind="ExternalInput")
    wv_ext = nc.dram_tensor("wvT", [C, 128], f32, kind="ExternalInput")
    bq_ext = nc.dram_tensor("bq", [128, 1], f32, kind="ExternalInput")
    bk_ext = nc.dram_tensor("bk", [128, 1], f32, kind="ExternalInput")
    bv_ext = nc.dram_tensor("bv", [128, 1], f32, kind="ExternalInput")
    pw_ext = nc.dram_tensor("pwT", [C, 128], bf16, kind="ExternalInput")
    pb_ext = nc.dram_tensor("pb", [128, 1], f32, kind="ExternalInput")
    nw_ext = nc.dram_tensor("nw", [C, 1], f32, kind="ExternalInput")
    nb_ext = nc.dram_tensor("nb", [C, 1], f32, kind="ExternalInput")
    gi_ext = nc.dram_tensor("gind", [NG, C], f32, kind="ExternalInput")
    giT_ext = nc.dram_tensor("gindT", [C, NG], f32, kind="ExternalInput")
    id_ext = nc.dram_tensor("identb", [128, 128], bf16, kind="ExternalInput")
    out_ext = nc.dram_tensor("out", [128, L], f32, kind="ExternalOutput")

    with tile.TileContext(nc, num_cores=N_CORES) as tc, ExitStack() as ctx:
        pers = ctx.enter_context(tc.tile_pool(name="pers", bufs=1))
        accp = ctx.enter_context(
            tc.tile_pool(name="accp", bufs=2, space="PSUM"))
        dram = ctx.enter_context(tc.tile_pool(name="dram", bufs=1,
                                              space="DRAM"))

        # ---- persistent tiles (DMAs for the big/late-use ones are emitted
        # inside emit_body, after the x loads, so they don't delay the
        # group-norm stats chain) ------------------------------------------
        wq_sb = [pers.tile([128, 128], f32, tag=f"wq{m}", name=f"wq{m}") for m in range(KT)]
        wk_sb = [pers.tile([128, 128], f32, tag=f"wk{m}", name=f"wk{m}") for m in range(KT)]
        wv_sb = [pers.tile([128, 128], f32, tag=f"wv{m}", name=f"wv{m}") for m in range(KT)]
        pw_sb = [pers.tile([128, 128], bf16, tag=f"pw{m}", name=f"pw{m}") for m in range(KT)]
        bq_sb = pers.tile([128, 1], f32, tag="bq")
        bk_sb = pers.tile([128, 1], f32, tag="bk")
        bv_sb = pers.tile([128, 1], f32, tag="bv")
        pb_sb = pers.tile([128, 1], f32, tag="pb")
        w_part = [pers.tile([128, 1], f32, tag=f"nw{m}", name=f"nw{m}") for m in range(KT)]
        b_part = [pers.tile([128, 1], f32, tag=f"nb{m}", name=f"nb{m}") for m in range(KT)]
        gi_sb = pers.tile([NG, C], f32, tag="gi")
        nc.sync.dma_start(gi_sb[:], gi_ext[:])
        giT_sb = [pers.tile([128, NG], f32, tag=f"giT{m}", name=f"giT{m}") for m in range(KT)]
        for m in range(KT):
            nc.sync.dma_start(giT_sb[m][:], giT_ext[128 * m:128 * (m + 1), :])
        id_sb = pers.tile([128, 128], bf16, tag="ident")
        ones_sb = pers.tile([128, 64], bf16, tag="ones")
        nc.gpsimd.memset(ones_sb[:], 1.0)

        xres_sb = pers.tile([128, L], f32, tag="xres")
        xres_pb = pers.tile([128, L], f32, tag="xrespb")

        def emit_pers_loads():
            for m in range(KT):
                nc.sync.dma_start(wq_sb[m][:], wq_ext[128 * m:128 * (m + 1), :])
                nc.sync.dma_start(wk_sb[m][:], wk_ext[128 * m:128 * (m + 1), :])
                nc.sync.dma_start(wv_sb[m][:], wv_ext[128 * m:128 * (m + 1), :])
                nc.sync.dma_start(pw_sb[m][:], pw_ext[128 * m:128 * (m + 1), :])
                nc.sync.dma_start(w_part[m][:], nw_ext[128 * m:128 * (m + 1), :])
                nc.sync.dma_start(b_part[m][:], nb_ext[128 * m:128 * (m + 1), :])
            nc.sync.dma_start(bq_sb[:], bq_ext[:])
            nc.sync.dma_start(bk_sb[:], bk_ext[:])
            nc.sync.dma_start(bv_sb[:], bv_ext[:])
            nc.sync.dma_start(pb_sb[:], pb_ext[:])
            nc.sync.dma_start(id_sb[:], id_ext[:])
            nc.sync.dma_start(xres_sb[:], xres_ext[:])
            # fold proj bias into the residual once
            nc.vector.tensor_scalar_add(xres_pb[:], xres_sb[:], pb_sb[:])

        # persistent activation tensors
        q_sb = pers.tile([128, L], bf16, tag="q")
        k_sb = pers.tile([128, L], bf16, tag="k")
        v_sb = pers.tile([128, L], bf16, tag="v")
        vT_sb = pers.tile([128, NS, 130], bf16, tag="vT")
        nc.gpsimd.memset(vT_sb[:, :, 64:65], 1.0)
        nc.gpsimd.memset(vT_sb[:, :, 129:130], 1.0)

        def emit_body(rep):
          with tc.tile_pool(name=f"xpool{rep}", bufs=1) as xpool:
              xs = [xpool.tile([128, L], bf16, tag=f"x{m}", name=f"x{m}") for m in range(KT)]
              stats = [xpool.tile([128, L // 512, 6], f32, tag=f"bs{m}", name=f"bs{m}")
                       for m in range(KT)]
              mv = [xpool.tile([128, 2], f32, tag=f"mv{m}", name=f"mv{m}") for m in range(KT)]
              rhs_m = [xpool.tile([128, 2], f32, tag=f"rh{m}", name=f"rh{m}")
                       for m in range(KT)]
              for m in range(KT):
                  for i in range(L // 512):
                      nc.sync.dma_start(
                          xs[m][:, 512 * i:512 * (i + 1)],
                          x_ext[128 * m:128 * (m + 1), 512 * i:512 * (i + 1)])
                      nc.vector.bn_stats(stats[m][:, i, :],
                                         xs[m][:, 512 * i:512 * (i + 1)])
                  nc.vector.bn_aggr(mv[m][:], stats[m][:])
              if rep == 0:
                  emit_pers_loads()
              for m in range(KT):
                  # rhs_m = [mean, var + mean^2]
                  nc.vector.tensor_copy(rhs_m[m][:, 0:1], mv[m][:, 0:1])
                  nc.vector.tensor_tensor(rhs_m[m][:, 1:2], mv[m][:, 0:1],
                                          mv[m][:, 0:1], op=OP.mult)
                  nc.vector.tensor_tensor(rhs_m[m][:, 1:2], rhs_m[m][:, 1:2],
                                          mv[m][:, 1:2], op=OP.add)
              # group sums over channels: [32, 2] = sum_m giT[m].T @ rhs_m
              st32 = accp.tile([NG, 2], f32, tag="aps")
              for m in range(KT):
                  nc.tensor.matmul(st32[:], giT_sb[m][:], rhs_m[m][:],
                                   start=(m == 0), stop=(m == KT - 1))
              gstat = xpool.tile([NG, 2], f32, tag="gstat")
              nc.vector.tensor_scalar_mul(gstat[:], st32[:], 1.0 / 16.0)
              gvar = xpool.tile([NG, 1], f32, tag="gvar")
              nc.vector.tensor_tensor(gvar[:], gstat[:, 0:1], gstat[:, 0:1],
                                      op=OP.mult)
              nc.vector.tensor_tensor(gvar[:], gstat[:, 1:2], gvar[:],
                                      op=OP.subtract)
              # rstd = exp(-0.5 * ln(var + eps))  (Rsqrt ACT is banned)
              eps_sb = xpool.tile([NG, 1], f32, tag="eps")
              nc.gpsimd.memset(eps_sb[:], EPS)
              glog = xpool.tile([NG, 1], f32, tag="glog")
              nc.scalar.activation(glog[:], gvar[:], AF.Ln, bias=eps_sb[:])
              rstd = xpool.tile([NG, 1], f32, tag="rstd")
              nc.scalar.activation(rstd[:], glog[:], AF.Exp, scale=-0.5)

              s_part = [xpool.tile([128, 1], f32, tag=f"sp{m}", name=f"sp{m}")
                        for m in range(KT)]
              t_part = [xpool.tile([128, 1], f32, tag=f"tp{m}", name=f"tp{m}")
                        for m in range(KT)]
              ttmp = [xpool.tile([128, 1], f32, tag=f"tt{m}", name=f"tt{m}")
                      for m in range(KT)]
              for m in range(KT):
                  rc = accp.tile([128, 1], f32, tag="aps")
                  nc.tensor.matmul(rc[:], gi_sb[:, 128 * m:128 * (m + 1)],
                                   rstd[:])
                  mc = accp.tile([128, 1], f32, tag="aps")
                  nc.tensor.matmul(mc[:], gi_sb[:, 128 * m:128 * (m + 1)],
                                   gstat[:, 0:1])
                  nc.vector.tensor_tensor(s_part[m][:], w_part[m][:], rc[:],
                                          op=OP.mult)
                  nc.vector.tensor_tensor(ttmp[m][:], mc[:], s_part[m][:],
                                          op=OP.mult)
                  nc.vector.tensor_tensor(t_part[m][:], b_part[m][:],
                                          ttmp[m][:], op=OP.subtract)

              # ---- fold group-norm into qkv weights + biases ---------------
              # q = (W.s) @ x + (b + W @ t), with W already SCALE-scaled on
              # the host for q/k.
              wqs = [xpool.tile([128, 128], bf16, tag=f"wqs{m}", name=f"wqs{m}")
                     for m in range(KT)]
              wks = [xpool.tile([128, 128], bf16, tag=f"wks{m}", name=f"wks{m}")
                     for m in range(KT)]
              wvs = [xpool.tile([128, 128], bf16, tag=f"wvs{m}", name=f"wvs{m}")
                     for m in range(KT)]
              for m in range(KT):
                  nc.vector.tensor_scalar_mul(wqs[m][:], wq_sb[m][:],
                                              s_part[m][:])
                  nc.vector.tensor_scalar_mul(wks[m][:], wk_sb[m][:],
                                              s_part[m][:])
                  nc.vector.tensor_scalar_mul(wvs[m][:], wv_sb[m][:],
                                              s_part[m][:])
              badj = {}
              for nm, wt, bias in (("q", wq_sb, bq_sb), ("k", wk_sb, bk_sb),
                                   ("v", wv_sb, bv_sb)):
                  bb = accp.tile([128, 1], f32, tag="aps")
                  for m in range(KT):
                      nc.tensor.matmul(bb[:], wt[m][:], t_part[m][:],
                                       start=(m == 0), stop=(m == KT - 1))
                  ba = xpool.tile([128, 1], f32, tag=f"badj{nm}")
                  nc.vector.tensor_tensor(ba[:], bias[:], bb[:], op=OP.add)
                  badj[nm] = ba

              # ---- qkv projections -----------------------------------------
              for wt, nm, dest in ((wqs, "q", q_sb),
                                   (wks, "k", k_sb),
                                   (wvs, "v", v_sb)):
                  for nn in range(NCH):
                      ps = accp.tile([128, 512], f32, tag="aps")
                      for m in range(KT):
                          nc.tensor.matmul(
                              ps[:], wt[m][:],
                              xs[m][:, 512 * nn:512 * (nn + 1)],
                              start=(m == 0), stop=(m == KT - 1))
                      nc.vector.tensor_scalar_add(
                          dest[:, 512 * nn:512 * (nn + 1)], ps[:],
                          badj[nm][:])

              # ---- v^T (with ones columns at 64 and 129) --------------------
              for s in range(NS):
                  tp = accp.tile([128, 128], bf16, tag="aps")
                  nc.tensor.transpose(tp[:], v_sb[:, 128 * s:128 * (s + 1)],
                                      id_sb[:])
                  nc.vector.tensor_copy(vT_sb[:, s, 0:64], tp[:, 0:64])
                  nc.vector.tensor_copy(vT_sb[:, s, 65:129], tp[:, 64:128])

          # ---- attention + gather + proj ---------------------------------
          NFLAT = 2 * NS
          groups = [list(range(i, min(i + GSZ, NFLAT)))
                      for i in range(0, NFLAT, GSZ)]
          a_tiles = {}

          with (
            tc.tile_pool(name=f"stp{rep}", bufs=2, space="PSUM") as stp,
            tc.tile_pool(name=f"expp{rep}", bufs=4) as expp,
            tc.tile_pool(name=f"attn{rep}", bufs=1) as attnp,
          ):
              # column segments of the AllGather: big early segments overlap
              # attention; the last segments are single t-blocks so the tail
              # only exposes one small collective + a small proj.
              SEGS = [(0, 4), (4, 6), (6, 7), (7, 8)]
              ag_queue = []

              def emit_ag(si):
                  s0, s1 = SEGS[si]
                  ncols = (s1 - s0) * TBLK
                  agin = dram.tile([128, ncols], bf16, tag=f"agin{rep}_{si}")
                  for j, tb in enumerate(range(s0, s1)):
                      ta, tbb = a_tiles[tb]
                      nc.sync.dma_start(
                          agin[0:64, TBLK * j:TBLK * (j + 1)], ta[:])
                      nc.sync.dma_start(
                          agin[64:128, TBLK * j:TBLK * (j + 1)], tbb[:])
                  agout = dram.tile([C, ncols], bf16, tag=f"agout{rep}_{si}")
                  if VARIANT == "nocoll":
                      nc.sync.dma_start(agout[0:128, :], agin[:])
                  else:
                      nc.gpsimd.collective_compute(
                          "AllGather", mybir.AluOpType.bypass,
                          replica_groups=[[0, 1, 2, 3], [4, 5, 6, 7]],
                          ins=[agin.opt()], outs=[agout.opt()])
                  return agout

              def emit_proj(si, agout):
                  s0, s1 = SEGS[si]
                  ncols = (s1 - s0) * TBLK
                  ag_sb = attnp.tile([128, KT, ncols], bf16, tag=f"ag{si}",
                                     bufs=1)
                  nc.sync.dma_start(
                      ag_sb[:], agout[:].rearrange("(a p) f -> p a f", p=128))
                  for nn in range(ncols // 512):
                      ps = accp.tile([128, 512], f32, tag="aps")
                      for m in range(KT):
                          nc.tensor.matmul(
                              ps[:], pw_sb[m][:],
                              ag_sb[:, m, 512 * nn:512 * (nn + 1)],
                              start=(m == 0), stop=(m == KT - 1))
                      osb = attnp.tile([128, 512], f32, tag="osb", bufs=2)
                      col = s0 * TBLK + 512 * nn
                      nc.vector.tensor_tensor(osb[:], ps[:],
                                              xres_pb[:, col:col + 512],
                                              op=OP.add)
                      nc.sync.dma_start(out_ext[:, col:col + 512], osb[:])

              a_ps_cur = {}

              def emit_st(tb, grp):
                  t0 = TBLK * tb
                  stg = stp.tile([128, GSZ, TBLK], f32, tag="st")
                  for idx, f in enumerate(grp):
                      s, hd = f // 2, f % 2
                      nc.tensor.matmul(
                          stg[:, idx, :],
                          k_sb[64 * hd:64 * (hd + 1), 128 * s:128 * (s + 1)],
                          q_sb[64 * hd:64 * (hd + 1), t0:t0 + TBLK])
                  eg = expp.tile([128, GSZ, TBLK], bf16, tag="eg")
                  nc.scalar.activation(eg[:, 0:len(grp), :],
                                       stg[:, 0:len(grp), :], AF.Exp)
                  return eg

              def emit_av(tb, grp, eg):
                  if tb not in a_ps_cur:
                      pA = accp.tile([65, TBLK], f32, tag="aps")
                      pB = accp.tile([65, TBLK], f32, tag="aps")
                      a_ps_cur[tb] = (pA, pB)
                  a_psA, a_psB = a_ps_cur[tb]
                  for idx, f in enumerate(grp):
                      s, hd = f // 2, f % 2
                      acc = a_psA if hd == 0 else a_psB
                      nc.tensor.matmul(
                          acc[:],
                          vT_sb[:, s, 65 * hd:65 * (hd + 1)],
                          eg[:, idx, :],
                          start=(s == 0), stop=(s == NS - 1),
                          skip_group_check=True)

              fin_mid = {}

              def emit_finalize_a(tb):
                  # normalize:  a[c,t] / denom[t];  denom sits in row 64.
                  # Copy everything out of the two aps PSUM slots FIRST so
                  # the next t-block's AV accumulators are not blocked
                  # behind the slow 1-lane reciprocal chain.
                  a_psA, a_psB = a_ps_cur.pop(tb)
                  dnA = attnp.tile([128, TBLK], f32, tag="rf", bufs=4)
                  nc.vector.tensor_copy(dnA[64:65, :], a_psA[64:65, :])
                  unA = attnp.tile([64, TBLK], f32, tag="un", bufs=2)
                  nc.vector.tensor_copy(unA[:], a_psA[0:64, :])
                  dnB = attnp.tile([128, TBLK], f32, tag="rf", bufs=4)
                  nc.vector.tensor_copy(dnB[64:65, :], a_psB[64:65, :])
                  unB = attnp.tile([64, TBLK], f32, tag="un", bufs=2)
                  nc.vector.tensor_copy(unB[:], a_psB[0:64, :])
                  # reciprocal on SBUF data (1-lane, slow, but off-PSUM);
                  # all ops stay on partition 64 (DVE lanes are fixed)
                  rfA = attnp.tile([128, TBLK], bf16, tag="rfb", bufs=4)
                  rfB = attnp.tile([128, TBLK], bf16, tag="rfb", bufs=4)
                  with nc.allow_low_precision(
                          reason="softmax denom reciprocal in bf16; "
                                 "0.4% rel err is well inside tolerance"):
                      nc.vector.reciprocal(rfA[64:65, :], dnA[64:65, :])
                      nc.vector.reciprocal(rfB[64:65, :], dnB[64:65, :])
                  fin_mid[tb] = (unA, unB, rfA, rfB)

              def emit_finalize_b(tb):
                  # deferred a couple of groups so the PE queue never waits
                  # on the reciprocal chain
                  unA, unB, rfA, rfB = fin_mid.pop(tb)
                  # broadcast tiles from the st pool (fast slot turnover)
                  rbA = stp.tile([64, TBLK], f32, tag="st")
                  nc.tensor.matmul(rbA[:], ones_sb[64:65, 0:64],
                                   rfA[64:65, :])
                  rbB = stp.tile([64, TBLK], f32, tag="st")
                  nc.tensor.matmul(rbB[:], ones_sb[64:65, 0:64],
                                   rfB[64:65, :])
                  ta = attnp.tile([64, TBLK], bf16, tag="an", bufs=2 * NTB)
                  nc.vector.tensor_tensor(ta[:], unA[:], rbA[:], op=OP.mult)
                  tbb = attnp.tile([64, TBLK], bf16, tag="an", bufs=2 * NTB)
                  nc.vector.tensor_tensor(tbb[:], unB[:], rbB[:], op=OP.mult)
                  a_tiles[tb] = (ta, tbb)
                  emit_post(tb)

              def emit_post(tb):
                  if VARIANT == "attnonly":
                      ta, tbb = a_tiles[tb]
                      col = TBLK * tb
                      nc.sync.dma_start(out_ext[0:64, col:col + TBLK], ta[:])
                      nc.sync.dma_start(out_ext[64:128, col:col + TBLK], tbb[:])
                      return
                  for si, (s0, s1) in enumerate(SEGS):
                      if tb == s1 - 1:
                          ag_queue.append((si, emit_ag(si), tb))
                  # run proj for segments whose gather has had >= 3 t-blocks
                  # of attention time to complete (so the proj's psum
                  # allocations never block the next AV accumulators while
                  # waiting for gather data)
                  for item in list(ag_queue):
                      si, agout, tb_emitted = item
                      if tb >= tb_emitted + 3:
                          emit_proj(si, agout)
                          ag_queue.remove(item)
                  if tb == NTB - 1:
                      for si, agout, _ in ag_queue:
                          emit_proj(si, agout)
                      ag_queue.clear()

              # software-pipelined by one group: the PE stream must not have
              # AV(g) (which waits on exp(g)) ahead of S^T(g+1), or the PE
              # stalls every group and the ACT engine (the bottleneck) idles.
              if VARIANT == "noattn":
                  for tb in range(NTB):
                      ta = attnp.tile([64, TBLK], bf16, tag="an",
                                      bufs=2 * NTB, name=f"za{tb}")
                      nc.gpsimd.memset(ta[:], 0.5)
                      tbb = attnp.tile([64, TBLK], bf16, tag="an",
                                       bufs=2 * NTB, name=f"zb{tb}")
                      nc.gpsimd.memset(tbb[:], 0.5)
                      a_tiles[tb] = (ta, tbb)
                      emit_post(tb)
              else:
                  flat = [(tb, grp) for tb in range(NTB) for grp in groups]
                  pend = None
                  pending_b = []   # [(tb, due_step)]
                  step = 0

                  def drain_b(step_now):
                      while pending_b and pending_b[0][1] <= step_now:
                          emit_finalize_b(pending_b.pop(0)[0])

                  for tb, grp in flat:
                      eg = emit_st(tb, grp)
                      step += 1
                      if pend is not None:
                          ptb, pgrp, peg = pend
                          emit_av(ptb, pgrp, peg)
                          if pgrp is groups[-1]:
                              emit_finalize_a(ptb)
                              pending_b.append((ptb, step + 2))
                      drain_b(step)
                      pend = (tb, grp, eg)
                  ptb, pgrp, peg = pend
                  emit_av(ptb, pgrp, peg)
                  emit_finalize_a(ptb)
                  pending_b.append((ptb, step))
                  drain_b(step + 10**9)

        for rep in range(REPS):
            emit_body(rep)

    nc.compile()
    return nc


def prep_in_maps(inputs, L=HW_L):
    x = np.asarray(inputs["x"], dtype=np.float32).reshape(B, C, L)
    qkv_w = np.asarray(inputs["qkv_w"], dtype=np.float32)
    qkv_b = np.asarray(inputs["qkv_b"], dtype=np.float32)
    proj_w = np.asarray(inputs["proj_w"], dtype=np.float32)
    proj_b = np.asarray(inputs["proj_b"], dtype=np.float32)
    norm_w = np.asarray(inputs["norm_w"], dtype=np.float32)
    norm_b = np.asarray(inputs["norm_b"], dtype=np.float32)

    gind = np.zeros((NG, C), dtype=np.float32)
    gind[np.arange(C) // 16, np.arange(C)] = 1.0
    identb = np.eye(128, dtype=np.float32).astype(BF16)

    def rows(h, kind):
        s = 192 * h + 64 * kind
        return slice(s, s + 64)

    in_maps = []
    for core in range(N_CORES):
        b, g = core // 4, core % 4
        hA, hB = 2 * g, 2 * g + 1
        wq = np.concatenate([qkv_w[rows(hA, 0)], qkv_w[rows(hB, 0)]], 0)
        wk = np.concatenate([qkv_w[rows(hA, 1)], qkv_w[rows(hB, 1)]], 0)
        wv = np.concatenate([qkv_w[rows(hA, 2)], qkv_w[rows(hB, 2)]], 0)
        bq = np.concatenate([qkv_b[rows(hA, 0)], qkv_b[rows(hB, 0)]])
        bk = np.concatenate([qkv_b[rows(hA, 1)], qkv_b[rows(hB, 1)]])
        bv = np.concatenate([qkv_b[rows(hA, 2)], qkv_b[rows(hB, 2)]])
        in_maps.append({
            "x": np.ascontiguousarray(x[b]).astype(BF16),
            "xres": np.ascontiguousarray(x[b, 128 * g:128 * (g + 1), :]),
            "wqT": np.ascontiguousarray((SCALE * wq).T),
            "wkT": np.ascontiguousarray((SCALE * wk).T),
            "wvT": np.ascontiguousarray(wv.T),
            "bq": np.ascontiguousarray((SCALE * bq).reshape(128, 1)),
            "bk": np.ascontiguousarray((SCALE * bk).reshape(128, 1)),
            "bv": np.ascontiguousarray(bv.reshape(128, 1)),
            "pwT": np.ascontiguousarray(
                proj_w[128 * g:128 * (g + 1), :].T).astype(BF16),
            "pb": np.ascontiguousarray(
                proj_b[128 * g:128 * (g + 1)].reshape(128, 1)),
            "nw": np.ascontiguousarray(norm_w.reshape(C, 1)),
            "nb": np.ascontiguousarray(norm_b.reshape(C, 1)),
            "gind": gind,
            "gindT": np.ascontiguousarray(gind.T),
            "identb": identb,
        })
    return in_maps


def gather_output(results, L=HW_L):
    out = np.empty((B, C, L), dtype=np.float32)
    for core in range(N_CORES):
        b, g = core // 4, core % 4
        out[b, 128 * g:128 * (g + 1), :] = results[core]["out"]
    s = int(np.sqrt(L))
    return out.reshape(B, C, s, s)


_NC_CACHE = {}


def get_nc(L=HW_L):
    if L not in _NC_CACHE:
        _NC_CACHE[L] = build_nc(L)
    return _NC_CACHE[L]


def kernel(**inputs):
    from concourse.bass_utils import run_bass_kernel_spmd

    nc = get_nc()
    in_maps = prep_in_maps(inputs)
    res = run_bass_kernel_spmd(nc, in_maps, core_ids=list(range(N_CORES)))
    return gather_output(res.results)


if __name__ == "__main__":
    rng = np.random.default_rng(0)
    inputs = {
        "x": rng.standard_normal((B, C, 64, 64), dtype=np.float32),
        "norm_w": rng.standard_normal(C, dtype=np.float32) * 0.1 + 1.0,
        "norm_b": rng.standard_normal(C, dtype=np.float32) * 0.1,
        "qkv_w": (rng.standard_normal((3 * C, C), dtype=np.float32)
                  / np.sqrt(C)),
        "qkv_b": rng.standard_normal(3 * C, dtype=np.float32) * 0.02,
        "proj_w": (rng.standard_normal((C, C), dtype=np.float32)
                   / np.sqrt(C)),
        "proj_b": rng.standard_normal(C, dtype=np.float32) * 0.02,
    }
    out = kernel(**inputs)
    print("kernel output", out.shape, out.dtype, float(np.abs(out).mean()))
